# revision 11
# baseline (speedup 1.0000x reference)
"""GAT encoder (10-layer, JK-concat) Trainium2 Bass kernel — 8-core node-parallel.

v2 design (batched):
  - Nodes globally degree-sorted, dealt round-robin across cores (rank r ->
    core r%8, slot r//8): every core's tile t spans the same global-degree
    window -> cross-core max-degree padding is minimal.
  - Tiles processed in groups of 4 (512-wide PSUM accumulation). Per-group
    batched DVE ops replace per-round instructions:
      dot:   tmp = gbuf * a_src_bcast ; asg = reduce3d(tmp)
      z      = asg + ad_bcast ; LeakyReLU ; exp (ACT)
      ewm    = ew * mask (per-group) ; S = outer-reduce ; rec = 1/(S+eps+wself)
      ewn    = ewm * rec_bcast ; gw = gbuf * ewn_expand (in-place)
      psum  += I @ gw[k-block]  (R_g matmuls, 512 cols each)
  - Attention alpha_own folded into the dense matmul: rhs = [W | Wa_s | Wa_d]
    (host-precomputed), giving h rows + alpha_src/alpha_dst in one PSUM tile.
  - bias dropped: PairNorm centering cancels any per-feature constant shift.
  - Final linear runs in the AllGather shadow.
"""

import numpy as np
import ml_dtypes
from contextlib import ExitStack

import concourse.bass as bass
import concourse.bacc as bacc
import concourse.tile as tile
import concourse.mybir as mybir

F32 = mybir.dt.float32
BF16 = mybir.dt.bfloat16
I16 = mybir.dt.int16
AX = mybir.AxisListType
OP = mybir.AluOpType
AF = mybir.ActivationFunctionType

N = 50000
E = 640000
IN = 128
HID = 128
L = 10
NC = 8
NSH = N // NC          # 6250
TILES = 49
NSHP = TILES * 128     # 6272
TBL = NSHP * NC        # 50176
HI_BASE = TBL - 32768  # 17408
NEG = 0.2
PEPS = 1e-5
SEPS = 1e-16
TPG = 2                # tiles per group


def preprocess(edge_index):
    src = np.asarray(edge_index[0], dtype=np.int64)
    dst = np.asarray(edge_index[1], dtype=np.int64)

    deg = np.bincount(dst, minlength=N)
    order = np.argsort(-deg, kind="stable")       # rank -> node
    outdeg = np.bincount(src, minlength=N)

    # Within each block of 8 same-in-degree-rank nodes, give the highest
    # OUT-degree nodes to the cores whose table row c*NSHP+s lands in the
    # int16 overlap window [HI_BASE, 32767] — their edges become lo/hi
    # flexible, shrinking the forced-split padding.
    blocks = order[:NSH * NC].reshape(NSH, NC)     # slot s -> its 8 nodes
    od = outdeg[blocks]
    perm = np.argsort(-od, axis=1, kind="stable")
    sorted_nodes = np.take_along_axis(blocks, perm, axis=1)
    core_order = np.empty((NSH, NC), np.int64)
    s_arr = np.arange(NSH)
    # window membership: c=3,4 always; c=5 iff s<1408; c=2 iff s>=4864
    reg0 = [3, 4, 5, 0, 1, 2, 6, 7]               # s < 1408
    reg1 = [3, 4, 0, 1, 2, 5, 6, 7]               # 1408 <= s < 4864
    reg2 = [3, 4, 2, 0, 1, 5, 6, 7]               # s >= 4864
    core_order[s_arr < 1408] = reg0
    core_order[(s_arr >= 1408) & (s_arr < 4864)] = reg1
    core_order[s_arr >= 4864] = reg2
    owner = np.empty(N, np.int64)
    slot = np.empty(N, np.int64)
    owner[sorted_nodes] = core_order
    slot[sorted_nodes] = s_arr[:, None]
    node_of = np.empty((NC, NSH), np.int64)
    node_of[owner[sorted_nodes.ravel()], slot[sorted_nodes.ravel()]] = sorted_nodes.ravel()
    row_of = owner * NSHP + slot                   # node -> table row

    # group tiles
    groups = [list(range(g, min(g + TPG, TILES))) for g in range(0, TILES, TPG)]

    # per (core, slot) lo/hi source-row lists
    ecore = owner[dst]
    eslot = slot[dst]
    esrow = row_of[src]
    lo_lists = [[None] * NSHP for _ in range(NC)]
    hi_lists = [[None] * NSHP for _ in range(NC)]
    for c in range(NC):
        m = ecore == c
        sl = eslot[m]
        rows = esrow[m]
        o = np.argsort(sl, kind="stable")
        sl = sl[o]
        rows = rows[o]
        counts = np.bincount(sl, minlength=NSHP)
        starts = np.concatenate([[0], np.cumsum(counts)])
        for p in range(NSHP):
            r = rows[starts[p]:starts[p + 1]]
            ml = r[r < HI_BASE]
            mh = r[r > 32767]
            fx = r[(r >= HI_BASE) & (r <= 32767)]
            nl, nh = len(ml), len(mh)
            lo_e, hi_e = [], []
            for v in fx:
                if nl <= nh:
                    lo_e.append(v); nl += 1
                else:
                    hi_e.append(v); nh += 1
            lo_lists[c][p] = np.concatenate([ml, np.array(lo_e, np.int64)]) if (len(ml) + len(lo_e)) else np.empty(0, np.int64)
            hi_lists[c][p] = (np.concatenate([mh, np.array(hi_e, np.int64)]) - HI_BASE) if (len(mh) + len(hi_e)) else np.empty(0, np.int64)

    # per-tile maxima over all cores/slots
    D_lo = np.zeros(TILES, np.int64)
    D_hi = np.zeros(TILES, np.int64)
    for c in range(NC):
        ll = np.array([len(x) for x in lo_lists[c]])
        hh = np.array([len(x) for x in hi_lists[c]])
        D_lo = np.maximum(D_lo, ll.reshape(TILES, 128).max(1))
        D_hi = np.maximum(D_hi, hh.reshape(TILES, 128).max(1))

    DG_lo = np.array([max(D_lo[t] for t in g) for g in groups])
    DG_hi = np.array([max(D_hi[t] for t in g) for g in groups])
    R = DG_lo + DG_hi
    NGR = len(groups)

    slots_tot = int(sum(int(R[g]) * len(groups[g]) * 128 for g in range(NGR)))
    pad_eff = slots_tot * NC / E

    # per-core idx (wrapped int16) and mask
    def wrap_idx(flat):
        n = len(flat)
        assert n % 16 == 0
        w = np.asarray(flat, np.int16).reshape(-1, 16).T
        return np.tile(w, (8, 1))

    # col offsets
    idx_off = {}          # (g, part) -> (col_start_in_idxall/16 units, n_idx)
    mask_off = {}         # g -> col start in maskall
    ic = 0
    mc = 0
    for g in range(NGR):
        tg = len(groups[g])
        idx_off[(g, "lo")] = (ic, int(DG_lo[g]) * tg * 128)
        ic += int(DG_lo[g]) * tg * 128 // 16
        idx_off[(g, "hi")] = (ic, int(DG_hi[g]) * tg * 128)
        ic += int(DG_hi[g]) * tg * 128 // 16
        mask_off[g] = mc
        mc += int(R[g]) * tg
    IDXCOLS = ic
    MASKCOLS = mc

    percore = []
    for c in range(NC):
        idxall = np.zeros((128, IDXCOLS), np.int16)
        maskall = np.zeros((128, MASKCOLS), np.float32)
        for g in range(NGR):
            tg = len(groups[g])
            dlo, dhi = int(DG_lo[g]), int(DG_hi[g])
            for part, dpart, lists in (("lo", dlo, lo_lists[c]), ("hi", dhi, hi_lists[c])):
                if dpart == 0:
                    continue
                flat = np.empty(dpart * tg * 128, np.int64)
                i = 0
                for k in range(dpart):
                    for j, t in enumerate(groups[g]):
                        base = t * 128
                        kk = k if part == "lo" else dlo + k
                        mrow = mask_off[g] + kk * tg + j
                        for sl in range(128):
                            lst = lists[base + sl]
                            if k < len(lst):
                                flat[i] = lst[k]
                                maskall[sl, mrow] = 1.0
                            else:
                                # pad fetch: spread across the table — same-row
                                # fetches serialize on one HBM bank (14x slower)
                                flat[i] = ((base + sl) * 97 + k * 12289) % 32000
                            i += 1
                col0, nidx = idx_off[(g, part)]
                assert nidx == len(flat)
                idxall[:, col0:col0 + nidx // 16] = wrap_idx(flat)
        percore.append({"idx": idxall, "mask": maskall})

    meta = {
        "groups": groups, "DG_lo": DG_lo, "DG_hi": DG_hi, "R": R,
        "idx_off": idx_off, "mask_off": mask_off,
        "IDXCOLS": IDXCOLS, "MASKCOLS": MASKCOLS,
        "order": order, "owner": owner, "slot": slot, "row_of": row_of,
        "node_of": node_of,
        "pad_eff": pad_eff, "slots_tot": slots_tot,
        "lo_lists": lo_lists, "hi_lists": hi_lists,
    }
    return meta, percore


# ---------------------------------------------------------------------------
# numpy emulator of the device computation (for logic validation)
# ---------------------------------------------------------------------------
def emulate(inputs, meta, percore, n_layers=L):
    bf = lambda a: a.astype(ml_dtypes.bfloat16).astype(np.float32)
    x = np.asarray(inputs["x"], np.float32)
    W0 = np.asarray(inputs["W0"], np.float32)
    Ws = np.asarray(inputs["Ws"], np.float32)
    att_src = np.asarray(inputs["att_src"], np.float32)
    att_dst = np.asarray(inputs["att_dst"], np.float32)
    lin_w = np.asarray(inputs["lin_w"], np.float32)
    lin_b = np.asarray(inputs["lin_b"], np.float32)

    groups = meta["groups"]
    DG_lo, DG_hi, R = meta["DG_lo"], meta["DG_hi"], meta["R"]
    order, row_of = meta["order"], meta["row_of"]
    NGR = len(groups)

    # per-core x slabs (rows = slots)
    Xc = np.zeros((NC, NSHP, IN), np.float32)
    node_of = np.zeros((NC, NSHP), np.int64)
    for c in range(NC):
        nodes = meta["node_of"][c]
        node_of[c, :NSH] = nodes
        Xc[c, :NSH] = bf(x[nodes])

    outfin = np.zeros((NC, NSHP, HID), np.float32)
    for l in range(n_layers):
        W = W0 if l == 0 else Ws[l - 1]
        Wext = np.concatenate([W, (W @ att_src[l])[:, None], (W @ att_dst[l])[:, None]], 1)
        Wext = bf(Wext)
        asrc_b = bf(att_src[l])

        # dense: h rows + alphas (bf16 rounded like device hrows_ext)
        HE = np.zeros((NC, NSHP, 130), np.float32)
        for c in range(NC):
            HE[c] = bf(Xc[c] @ Wext)

        # table: core blocks stacked
        table = bf(np.concatenate([HE[c][:, :128] for c in range(NC)], 0))  # [TBL,128]

        for c in range(NC):
            he = HE[c]
            as_own = he[:, 128]
            ad_own = he[:, 129]
            zs = as_own + ad_own
            wself = np.exp(np.where(zs > 0, zs, NEG * zs))
            mask = percore[c]["mask"]
            idxall = percore[c]["idx"]
            rows_out = np.zeros((NSHP, HID), np.float32)
            for g in range(NGR):
                tg = len(groups[g])
                dlo, dhi = int(DG_lo[g]), int(DG_hi[g])
                rg = int(R[g])
                # reconstruct gathered buffer from idx
                gbuf = np.zeros((128, rg * tg, 128), np.float32)
                for part, dpart, base_k, tb in (("lo", dlo, 0, 0), ("hi", dhi, dlo, HI_BASE)):
                    if dpart == 0:
                        continue
                    col0, nidx = meta["idx_off"][(g, part)]
                    wrapped = idxall[:16, col0:col0 + nidx // 16]
                    flat = wrapped.T.reshape(-1)          # undo wrap
                    rows = flat.astype(np.int64) + tb
                    gg = table[rows]                       # [n, 128]
                    gg = gg.reshape(dpart, tg, 128, 128)   # [k, j, sl, f]
                    for k in range(dpart):
                        for j in range(tg):
                            gbuf[:, (base_k + k) * tg + j, :] = gg[k, j]
                # dot
                tmp = bf(gbuf * asrc_b[None, None, :])
                asg = tmp.sum(2)                           # [128, rg*tg]
                # z = asg + ad (per tile j)
                ad_g = np.stack([ad_own[t * 128:(t + 1) * 128] for t in groups[g]], 1)  # [128, tg]
                z = asg.reshape(128, rg, tg) + ad_g[:, None, :]
                z = z.reshape(128, rg * tg)
                zl = np.where(z > 0, z, NEG * z)
                ew = np.exp(zl)
                m = mask[:, meta["mask_off"][g]:meta["mask_off"][g] + rg * tg]
                ewm = ew * m
                S = ewm.reshape(128, rg, tg).sum(1)        # [128, tg]
                ws_g = np.stack([wself[t * 128:(t + 1) * 128] for t in groups[g]], 1)
                rec = 1.0 / (S + SEPS + ws_g)
                ewn = bf(ewm * np.repeat(rec[:, None, :], rg, 1).reshape(128, rg * tg))
                gw = bf(gbuf * ewn[:, :, None])
                pa = gw.reshape(128, rg, tg, 128).sum(1)   # [128, tg, 128]
                wsn = ws_g * rec
                for j, t in enumerate(groups[g]):
                    h_own = he[t * 128:(t + 1) * 128, :128]
                    rows_out[t * 128:(t + 1) * 128] = pa[:, j] + h_own * wsn[:, j:j + 1]
            rows_out[NSH:] = 0.0
            HE[c, :, :128] = rows_out                      # reuse: pre-pairnorm rows
        # pairnorm stats (global)
        allrows = np.stack([bf(HE[c][:, :128]) for c in range(NC)])   # bf16 slab
        fsum = allrows.sum((0, 1))
        sq = (bf(allrows * allrows)).sum()
        mu = fsum / N
        var = PEPS + sq / N - (mu * mu).sum()
        invd = 1.0 / np.sqrt(var)
        for c in range(NC):
            xx = bf(HE[c][:, :128])
            g = (xx - mu[None, :]) * invd
            from scipy.special import erf
            gel = 0.5 * g * (1 + erf(g / np.sqrt(2)))
            gel[NSH:] = 0.0
            Xc[c] = bf(gel)
            outfin[c] += bf(Xc[c]) @ bf(lin_w[l * HID:(l + 1) * HID])

    out = np.empty((N, HID), np.float32)
    for c in range(NC):
        out[node_of[c, :NSH]] = outfin[c, :NSH] + lin_b[None, :]
    return out


def build(nc, meta, n_layers=L, sim_safe=False):
    groups = meta["groups"]
    DG_lo, DG_hi, R = meta["DG_lo"], meta["DG_hi"], meta["R"]
    idx_off, mask_off = meta["idx_off"], meta["mask_off"]
    IDXCOLS, MASKCOLS = meta["IDXCOLS"], meta["MASKCOLS"]
    NGR = len(groups)
    RMAX = int(R.max())
    TMPW = max(RMAX * TPG * 128, NSHP)
    GELU = AF.Sigmoid if sim_safe else AF.Gelu

    xT_in = nc.dram_tensor("xT", [128, NSHP], BF16, kind="ExternalInput")
    idx_in = nc.dram_tensor("idx", [128, IDXCOLS], I16, kind="ExternalInput")
    mask_in = nc.dram_tensor("mask", [128, MASKCOLS], BF16, kind="ExternalInput")
    wext_in = nc.dram_tensor("Wext", [n_layers, 128, 130], BF16, kind="ExternalInput")
    asrc_in = nc.dram_tensor("asrc", [n_layers, 128, 128], BF16, kind="ExternalInput")
    linw_in = nc.dram_tensor("linw", [n_layers, 128, 128], BF16, kind="ExternalInput")
    linbr_in = nc.dram_tensor("linbr", [128, 128], F32, kind="ExternalInput")
    identb_in = nc.dram_tensor("identb", [128, 128], BF16, kind="ExternalInput")
    ones_in = nc.dram_tensor("ones", [128, 128], F32, kind="ExternalInput")
    padm_in = nc.dram_tensor("padm", [128, 1], F32, kind="ExternalInput")
    y_out = nc.dram_tensor("y", [NSHP, 128], F32, kind="ExternalOutput")

    ag_in = nc.dram_tensor("ag_in", [NSHP, 128], BF16)
    table = nc.dram_tensor("table", [TBL, 128], BF16, addr_space="Shared")
    st_in = nc.dram_tensor("st_in", [128, 2], F32)
    st_out = nc.dram_tensor("st_out", [NC * 128, 2], F32, addr_space="Shared")

    RG = [list(range(NC))]

    with tile.TileContext(nc) as tc, ExitStack() as ctx:
        P = ctx.enter_context(tc.tile_pool(name="persist", bufs=1))
        xT = P.tile([128, NSHP], BF16, tag="xT")
        he = P.tile([128, TILES * 130], BF16, tag="he")
        outfin = P.tile([128, NSHP], F32, tag="outfin")
        idx_sb = P.tile([128, IDXCOLS], I16, tag="idx")
        mask_sb = P.tile([128, MASKCOLS], BF16, tag="mask")
        wext_sb = P.tile([128, n_layers * 130], BF16, tag="Wext")
        asrc_sb = P.tile([128, n_layers * 128], BF16, tag="asrc")
        linw_sb = P.tile([128, n_layers * 128], BF16, tag="linw")
        linbr_sb = P.tile([128, 128], F32, tag="linbr")
        identb = P.tile([128, 128], BF16, tag="identb")
        ones_sb = P.tile([128, 128], F32, tag="ones")
        padm_sb = P.tile([128, 1], F32, tag="padm")
        wself = P.tile([128, TILES], F32, tag="wself")
        stp = P.tile([128, 2], F32, tag="stp")
        gtmp = P.tile([128, 16], F32, tag="gtmp")
        gstat = P.tile([128, 2], F32, tag="gstat")

        nc.sync.dma_start(idx_sb[:], idx_in.ap())
        nc.sync.dma_start(mask_sb[:], mask_in.ap())
        nc.sync.dma_start(wext_sb[:].rearrange("a (l b) -> a l b", b=130), wext_in.ap().rearrange("l a b -> a l b"))
        nc.sync.dma_start(asrc_sb[:].rearrange("a (l b) -> a l b", b=128), asrc_in.ap().rearrange("l a b -> a l b"))
        nc.sync.dma_start(linw_sb[:].rearrange("a (l b) -> a l b", b=128), linw_in.ap().rearrange("l a b -> a l b"))
        nc.sync.dma_start(linbr_sb[:], linbr_in.ap())
        nc.sync.dma_start(identb[:], identb_in.ap())
        nc.sync.dma_start(ones_sb[:], ones_in.ap())
        nc.sync.dma_start(padm_sb[:], padm_in.ap())
        nc.sync.dma_start(xT[:], xT_in.ap())

        PD = ctx.enter_context(tc.tile_pool(name="pd", bufs=2, space="PSUM"))
        PT = ctx.enter_context(tc.tile_pool(name="pt", bufs=2, space="PSUM"))
        PA = ctx.enter_context(tc.tile_pool(name="pa", bufs=2, space="PSUM"))
        PM = ctx.enter_context(tc.tile_pool(name="pm", bufs=2, space="PSUM"))
        GB = ctx.enter_context(tc.tile_pool(name="gb", bufs=2))
        WK = ctx.enter_context(tc.tile_pool(name="wk", bufs=1))
        SC = ctx.enter_context(tc.tile_pool(name="sc", bufs=3))
        SS = ctx.enter_context(tc.tile_pool(name="ss", bufs=4))

        he3 = he[:].rearrange("p (t c) -> p t c", c=130)

        for l in range(n_layers):
            wxl = wext_sb[:, l * 130:(l + 1) * 130]
            asl = asrc_sb[:, l * 128:(l + 1) * 128]
            lwl = linw_sb[:, l * 128:(l + 1) * 128]

            # ---- dense: h rows + alpha_src/alpha_dst per tile
            for t in range(TILES):
                pd = PD.tile([128, 130], F32, tag="pd")
                nc.tensor.matmul(pd[:], xT[:, t * 128:(t + 1) * 128], wxl,
                                 start=True, stop=True)
                nc.scalar.activation(he[:, t * 130:(t + 1) * 130], pd[:], AF.Copy)

            # ---- self weight: wself = exp(LR(as+ad))
            zs = SC.tile([128, TILES], F32, tag="zs")
            nc.vector.tensor_tensor(zs[:], he3[:, :, 128], he3[:, :, 129], op=OP.add)
            zs2 = SC.tile([128, TILES], F32, tag="zs2")
            nc.vector.scalar_tensor_tensor(out=zs2[:], in0=zs[:], scalar=NEG,
                                           in1=zs[:], op0=OP.mult, op1=OP.max)
            nc.scalar.activation(wself[:], zs2[:], AF.Exp)

            # ---- AllGather h rows
            nc.sync.dma_start(ag_in.ap().rearrange("(t p) f -> p t f", p=128),
                              he3[:, :, 0:128])
            nc.gpsimd.collective_compute(
                "AllGather", OP.bypass, replica_groups=RG,
                ins=[ag_in.ap()], outs=[table.ap()])

            # ---- final-linear increment for the PREVIOUS layer's output
            # (xT currently holds outs[l-1]; runs in the AG shadow)
            if l > 0:
                lwp = linw_sb[:, (l - 1) * 128:l * 128]
                for t in range(TILES):
                    pf = PM.tile([128, 128], F32, tag="pm")
                    nc.tensor.matmul(pf[:], xT[:, t * 128:(t + 1) * 128], lwp,
                                     start=True, stop=True)
                    tsl = slice(t * 128, (t + 1) * 128)
                    if l == 1:
                        nc.vector.tensor_copy(outfin[:, tsl], pf[:])
                    else:
                        nc.vector.tensor_tensor(outfin[:, tsl], outfin[:, tsl], pf[:], op=OP.add)

            # ---- aggregation per group
            for g in range(NGR):
                tg = len(groups[g])
                dlo, dhi = int(DG_lo[g]), int(DG_hi[g])
                rg = dlo + dhi
                ncols = rg * tg
                gbuf = GB.tile([128, RMAX * TPG * 128], BF16, tag="gb")
                g3 = gbuf[:].rearrange("p (c f) -> p c f", f=128)
                if dlo:
                    c0, nidx = idx_off[(g, "lo")]
                    nc.gpsimd.dma_gather(
                        g3[:, :dlo * tg, :], table.ap()[:32768, :],
                        idx_sb[:, c0:c0 + nidx // 16], nidx, nidx, 128,
                        single_packet=False)
                if dhi:
                    c0, nidx = idx_off[(g, "hi")]
                    nc.gpsimd.dma_gather(
                        g3[:, dlo * tg:ncols, :], table.ap()[HI_BASE:TBL, :],
                        idx_sb[:, c0:c0 + nidx // 16], nidx, nidx, 128,
                        single_packet=False)

                wk = WK.tile([128, TMPW], BF16, tag="wk")
                w3 = wk[:].rearrange("p (c f) -> p c f", f=128)
                # dot: asg[p, c] = sum_f gbuf[p,c,f]*a[f]
                a_b = asl.unsqueeze(1).broadcast_to([128, ncols, 128])
                nc.vector.tensor_tensor(w3[:, :ncols, :], g3[:, :ncols, :], a_b, op=OP.mult)
                asg = SC.tile([128, RMAX * TPG], F32, tag="asg")
                nc.vector.tensor_reduce(out=asg[:, :ncols], in_=w3[:, :ncols, :],
                                        axis=AX.X, op=OP.add)
                # z = asg + ad_bcast ; LeakyReLU ; exp
                ad_b = he3[:, groups[g][0]:groups[g][0] + tg, 129] \
                    .unsqueeze(1).broadcast_to([128, rg, tg])
                zg = SC.tile([128, RMAX * TPG], F32, tag="zg")
                nc.vector.tensor_tensor(zg[:, :ncols].rearrange("p (k j) -> p k j", j=tg),
                                        asg[:, :ncols].rearrange("p (k j) -> p k j", j=tg),
                                        ad_b, op=OP.add)
                zg2 = SC.tile([128, RMAX * TPG], F32, tag="zg2")
                nc.vector.scalar_tensor_tensor(out=zg2[:, :ncols], in0=zg[:, :ncols],
                                               scalar=NEG, in1=zg[:, :ncols],
                                               op0=OP.mult, op1=OP.max)
                ew = SC.tile([128, RMAX * TPG], F32, tag="ew")
                nc.scalar.activation(ew[:, :ncols], zg2[:, :ncols], AF.Exp)
                # masked weights + per-tile sums
                ewm = SC.tile([128, RMAX * TPG], F32, tag="ewm")
                mo = mask_off[g]
                nc.vector.tensor_tensor(ewm[:, :ncols], ew[:, :ncols],
                                        mask_sb[:, mo:mo + ncols], op=OP.mult)
                S = SS.tile([128, TPG], F32, tag="S")
                nc.vector.tensor_reduce(out=S[:, :tg],
                                        in_=ewm[:, :ncols].rearrange("p (k j) -> p j k", j=tg),
                                        axis=AX.X, op=OP.add)
                ws_g = wself[:, groups[g][0]:groups[g][0] + tg]
                Sp = SS.tile([128, TPG], F32, tag="Sp")
                nc.vector.scalar_tensor_tensor(out=Sp[:, :tg], in0=S[:, :tg], scalar=SEPS,
                                               in1=ws_g, op0=OP.add, op1=OP.add)
                rec = SS.tile([128, TPG], F32, tag="rec")
                nc.vector.reciprocal(rec[:, :tg], Sp[:, :tg])
                # normalized weights, expanded apply
                ewn = SC.tile([128, RMAX * TPG], BF16, tag="ewn")
                rec_b = rec[:, :tg].unsqueeze(1).broadcast_to([128, rg, tg])
                nc.vector.tensor_tensor(ewn[:, :ncols].rearrange("p (k j) -> p k j", j=tg),
                                        ewm[:, :ncols].rearrange("p (k j) -> p k j", j=tg),
                                        rec_b, op=OP.mult)
                ewn_e = ewn[:, :ncols].unsqueeze(-1).broadcast_to([128, ncols, 128])
                nc.vector.tensor_tensor(w3[:, :ncols, :], g3[:, :ncols, :], ewn_e, op=OP.mult)
                # accumulate rounds
                pa = PA.tile([128, TPG * 128], F32, tag="pa")
                for k in range(rg):
                    nc.tensor.matmul(pa[:, :tg * 128], identb[:],
                                     wk[:, k * tg * 128:(k + 1) * tg * 128],
                                     start=(k == 0), stop=(k == rg - 1))
                # self term + normalize-free combine
                wsn = SS.tile([128, TPG], F32, tag="wsn")
                nc.vector.tensor_tensor(wsn[:, :tg], ws_g, rec[:, :tg], op=OP.mult)
                hslab = he3[:, groups[g][0]:groups[g][0] + tg, 0:128]
                selft = SC.tile([128, TPG * 128], F32, tag="selft")
                wsn_b = wsn[:, :tg].unsqueeze(-1).broadcast_to([128, tg, 128])
                nc.vector.tensor_tensor(selft[:, :tg * 128].rearrange("p (j f) -> p j f", f=128),
                                        hslab, wsn_b, op=OP.mult)
                rows = SC.tile([128, TPG * 128], BF16, tag="rows")
                nc.vector.scalar_tensor_tensor(out=rows[:, :tg * 128], in0=pa[:, :tg * 128],
                                               scalar=1.0, in1=selft[:, :tg * 128],
                                               op0=OP.mult, op1=OP.add)
                if groups[g][-1] == TILES - 1:
                    lastj = (TILES - 1) - groups[g][0]
                    nc.vector.tensor_scalar_mul(rows[:, lastj * 128:(lastj + 1) * 128],
                                                rows[:, lastj * 128:(lastj + 1) * 128],
                                                padm_sb[:])
                # transpose into xT slab (pre-pairnorm)
                for j, t in enumerate(groups[g]):
                    pt = PT.tile([128, 128], BF16, tag="pt")
                    nc.tensor.transpose(pt[:], rows[:, j * 128:(j + 1) * 128], identb[:])
                    nc.scalar.activation(xT[:, t * 128:(t + 1) * 128], pt[:], AF.Copy)

            # ---- pairnorm stats
            fsum = SS.tile([128, 1], F32, tag="fsum")
            nc.vector.tensor_reduce(out=fsum[:], in_=xT[:], axis=AX.X, op=OP.add)
            wk = WK.tile([128, TMPW], BF16, tag="wk")
            sqc = SS.tile([128, 1], F32, tag="sqc")
            nc.scalar.activation(wk[:, :NSHP], xT[:], AF.Square, accum_out=sqc[:])
            nc.vector.tensor_copy(stp[:, :1], fsum[:])
            nc.vector.tensor_copy(stp[:, 1:2], sqc[:])
            nc.sync.dma_start(st_in.ap(), stp[:])
            nc.gpsimd.collective_compute(
                "AllGather", OP.bypass, replica_groups=RG,
                ins=[st_in.ap()], outs=[st_out.ap()])
            nc.sync.dma_start(gtmp[:].rearrange("p (r c) -> p r c", c=2),
                              st_out.ap().rearrange("(r p) c -> p r c", p=128))
            nc.vector.tensor_reduce(out=gstat[:],
                                    in_=gtmp[:].rearrange("p (r c) -> p c r", c=2),
                                    axis=AX.X, op=OP.add)
            mu = SS.tile([128, 1], F32, tag="mu")
            nc.vector.tensor_scalar_mul(mu[:], gstat[:, :1], 1.0 / N)
            st2 = SS.tile([128, 2], F32, tag="st2")
            nc.vector.tensor_copy(st2[:, :1], gstat[:, 1:2])
            nc.vector.tensor_tensor(st2[:, 1:2], mu[:], mu[:], op=OP.mult)
            p2 = PM.tile([128, 128], F32, tag="pm")
            nc.tensor.matmul(p2[:1, :2], ones_sb[:, :1], st2[:], start=True, stop=True)
            tot = SS.tile([1, 2], F32, tag="tot")
            nc.vector.tensor_copy(tot[:], p2[:1, :2])
            v3 = SS.tile([1, 1], F32, tag="v3")
            nc.vector.tensor_scalar(v3[:], tot[:, :1], 1.0 / N, PEPS,
                                    op0=OP.mult, op1=OP.add)
            v4 = SS.tile([1, 1], F32, tag="v4")
            nc.vector.tensor_tensor(v4[:], v3[:], tot[:, 1:2], op=OP.subtract)
            den = SS.tile([1, 1], F32, tag="den")
            nc.scalar.activation(den[:], v4[:], AF.Sqrt)
            invd = SS.tile([1, 1], F32, tag="invd")
            nc.vector.reciprocal(invd[:], den[:])
            pb1 = PM.tile([128, 128], F32, tag="pm")
            nc.tensor.matmul(pb1[:, :1], ones_sb[:1, :], invd[:], start=True, stop=True)
            invdr = SS.tile([128, 1], F32, tag="invdr")
            nc.vector.tensor_copy(invdr[:], pb1[:, :1])
            nms = SS.tile([128, 1], F32, tag="nms")
            nc.vector.tensor_scalar(nms[:], mu[:], invdr[:], -1.0,
                                    op0=OP.mult, op1=OP.mult)

            # ---- pairnorm + gelu fused on ACT (in-place on xT)
            for ch0 in range(0, NSHP, 512):
                chsz = min(512, NSHP - ch0)
                csl = slice(ch0, ch0 + chsz)
                nc.scalar.activation(xT[:, csl], xT[:, csl], GELU,
                                     bias=nms[:], scale=invdr[:])

        # ---- final-linear for last layer + lin_b + output
        lwl = linw_sb[:, (n_layers - 1) * 128:n_layers * 128]
        for t in range(TILES):
            pf = PM.tile([128, 128], F32, tag="pm")
            nc.tensor.matmul(pf[:], xT[:, t * 128:(t + 1) * 128], lwl,
                             start=True, stop=True)
            tsl = slice(t * 128, (t + 1) * 128)
            if n_layers == 1:
                nc.vector.tensor_copy(outfin[:, tsl], pf[:])
            else:
                nc.vector.tensor_tensor(outfin[:, tsl], outfin[:, tsl], pf[:], op=OP.add)
        lb_b = linbr_sb[:].unsqueeze(1).broadcast_to([128, TILES, 128])
        nc.vector.tensor_tensor(outfin[:].rearrange("p (t f) -> p t f", f=128),
                                outfin[:].rearrange("p (t f) -> p t f", f=128),
                                lb_b, op=OP.add)
        nc.sync.dma_start(y_out.ap().rearrange("(t p) f -> p t f", p=128),
                          outfin[:].rearrange("p (t f) -> p t f", f=128))

    return nc


def make_inputs(inputs, meta, percore, n_layers=L):
    x = np.asarray(inputs["x"], np.float32)
    W0 = np.asarray(inputs["W0"], np.float32)
    Ws = np.asarray(inputs["Ws"], np.float32)
    att_src = np.asarray(inputs["att_src"], np.float32)
    att_dst = np.asarray(inputs["att_dst"], np.float32)
    lin_w = np.asarray(inputs["lin_w"], np.float32)
    lin_b = np.asarray(inputs["lin_b"], np.float32)
    node_of = meta["node_of"]

    wext = np.zeros((n_layers, 128, 130), np.float32)
    for l in range(n_layers):
        W = W0 if l == 0 else Ws[l - 1]
        wext[l, :, :128] = W
        wext[l, :, 128] = W @ att_src[l]
        wext[l, :, 129] = W @ att_dst[l]
    wext = wext.astype(ml_dtypes.bfloat16)
    asrc = np.stack([np.tile(att_src[i], (128, 1)) for i in range(n_layers)]).astype(ml_dtypes.bfloat16)
    linw = np.stack([lin_w[i * HID:(i + 1) * HID] for i in range(n_layers)]).astype(ml_dtypes.bfloat16)
    linbr = np.tile(lin_b, (128, 1)).astype(np.float32)
    identb = np.eye(128, dtype=ml_dtypes.bfloat16)
    ones = np.ones((128, 128), np.float32)
    padm = np.zeros((128, 1), np.float32)
    padm[:NSH - (TILES - 1) * 128] = 1.0

    in_maps = []
    for c in range(NC):
        nodes = node_of[c]
        xT = np.zeros((128, NSHP), ml_dtypes.bfloat16)
        xT[:, :NSH] = x[nodes].astype(ml_dtypes.bfloat16).T
        in_maps.append({
            "xT": xT, "idx": percore[c]["idx"],
            "mask": percore[c]["mask"].astype(ml_dtypes.bfloat16),
            "Wext": wext, "asrc": asrc, "linw": linw, "linbr": linbr,
            "identb": identb, "ones": ones, "padm": padm,
        })
    return in_maps


def assemble_output(results, meta):
    node_of = meta["node_of"]
    out = np.empty((N, HID), np.float32)
    for c in range(NC):
        out[node_of[c]] = results[c]["y"][:NSH]
    return out


_CACHE = {}


def _get_compiled(edge_key, edge_index):
    if edge_key not in _CACHE:
        meta, percore = preprocess(edge_index)
        nc = bacc.Bacc("TRN2", target_bir_lowering=False, debug=False,
                       num_devices=NC)
        build(nc, meta, n_layers=L, sim_safe=False)
        nc.compile()
        _CACHE[edge_key] = (nc, meta, percore)
    return _CACHE[edge_key]


def kernel(**inputs):
    from concourse.bass_utils import run_bass_kernel_spmd
    edge_index = np.asarray(inputs["edge_index"])
    edge_key = hash(edge_index.tobytes())
    nc, meta, percore = _get_compiled(edge_key, edge_index)
    in_maps = make_inputs(inputs, meta, percore, n_layers=L)
    res = run_bass_kernel_spmd(nc, in_maps, list(range(NC)))
    return assemble_output(res.results, meta)


if __name__ == "__main__":
    import sys, time, pickle, os
    sys.path.insert(0, "/root/problem")
    import jax
    import reference
    cpu = jax.devices("cpu")[0]
    with jax.default_device(cpu):
        inputs = {k: np.asarray(v) for k, v in reference.setup_inputs().items()}
    t0 = time.time()
    meta, percore = preprocess(inputs["edge_index"])
    print("preprocess:", round(time.time() - t0, 1), "s; pad_eff:", round(meta["pad_eff"], 3),
          "slots_tot:", meta["slots_tot"], "R:", meta["R"].tolist())
    if "--emulate" in sys.argv:
        t0 = time.time()
        got = emulate(inputs, meta, percore)
        print("emulate:", round(time.time() - t0, 1), "s")
        with jax.default_device(cpu):
            exp = np.asarray(reference.reference(**inputs))
        rel = np.linalg.norm(got - exp) / np.linalg.norm(exp)
        print("emulator rel-l2 vs reference:", rel)
    if "--build" in sys.argv:
        t0 = time.time()
        nc = bacc.Bacc("TRN2", target_bir_lowering=False, debug=False, num_devices=NC)
        build(nc, meta, n_layers=L, sim_safe=False)
        nc.compile()
        print("build+compile:", round(time.time() - t0, 1), "s")
        f = nc.m.functions[0]
        print("instructions:", sum(len(b.instructions) for b in f.blocks))


# revision 12
# speedup vs baseline: 1.4236x; 1.4236x over previous
"""GAT encoder (10-layer, JK-concat) Trainium2 Bass kernel — 8-core node-parallel.

v2 design (batched):
  - Nodes globally degree-sorted, dealt round-robin across cores (rank r ->
    core r%8, slot r//8): every core's tile t spans the same global-degree
    window -> cross-core max-degree padding is minimal.
  - Tiles processed in groups of 4 (512-wide PSUM accumulation). Per-group
    batched DVE ops replace per-round instructions:
      dot:   tmp = gbuf * a_src_bcast ; asg = reduce3d(tmp)
      z      = asg + ad_bcast ; LeakyReLU ; exp (ACT)
      ewm    = ew * mask (per-group) ; S = outer-reduce ; rec = 1/(S+eps+wself)
      ewn    = ewm * rec_bcast ; gw = gbuf * ewn_expand (in-place)
      psum  += I @ gw[k-block]  (R_g matmuls, 512 cols each)
  - Attention alpha_own folded into the dense matmul: rhs = [W | Wa_s | Wa_d]
    (host-precomputed), giving h rows + alpha_src/alpha_dst in one PSUM tile.
  - bias dropped: PairNorm centering cancels any per-feature constant shift.
  - Final linear runs in the AllGather shadow.
"""

import numpy as np
import ml_dtypes
from contextlib import ExitStack

import concourse.bass as bass
import concourse.bacc as bacc
import concourse.tile as tile
import concourse.mybir as mybir

F32 = mybir.dt.float32
BF16 = mybir.dt.bfloat16
I16 = mybir.dt.int16
AX = mybir.AxisListType
OP = mybir.AluOpType
AF = mybir.ActivationFunctionType

N = 50000
E = 640000
IN = 128
HID = 128
L = 10
NC = 8
NSH = N // NC          # 6250
TILES = 49
NSHP = TILES * 128     # 6272
TBL = NSHP * NC        # 50176
HI_BASE = TBL - 32768  # 17408
NEG = 0.2
PEPS = 1e-5
SEPS = 1e-16
TPG = 4                # tiles per group


def preprocess(edge_index):
    src = np.asarray(edge_index[0], dtype=np.int64)
    dst = np.asarray(edge_index[1], dtype=np.int64)

    deg = np.bincount(dst, minlength=N)
    order = np.argsort(-deg, kind="stable")       # rank -> node
    outdeg = np.bincount(src, minlength=N)

    # Within each block of 8 same-in-degree-rank nodes, give the highest
    # OUT-degree nodes to the cores whose table row c*NSHP+s lands in the
    # int16 overlap window [HI_BASE, 32767] — their edges become lo/hi
    # flexible, shrinking the forced-split padding.
    blocks = order[:NSH * NC].reshape(NSH, NC)     # slot s -> its 8 nodes
    od = outdeg[blocks]
    perm = np.argsort(-od, axis=1, kind="stable")
    sorted_nodes = np.take_along_axis(blocks, perm, axis=1)
    core_order = np.empty((NSH, NC), np.int64)
    s_arr = np.arange(NSH)
    # window membership: c=3,4 always; c=5 iff s<1408; c=2 iff s>=4864
    reg0 = [3, 4, 5, 0, 1, 2, 6, 7]               # s < 1408
    reg1 = [3, 4, 0, 1, 2, 5, 6, 7]               # 1408 <= s < 4864
    reg2 = [3, 4, 2, 0, 1, 5, 6, 7]               # s >= 4864
    core_order[s_arr < 1408] = reg0
    core_order[(s_arr >= 1408) & (s_arr < 4864)] = reg1
    core_order[s_arr >= 4864] = reg2
    owner = np.empty(N, np.int64)
    slot = np.empty(N, np.int64)
    owner[sorted_nodes] = core_order
    slot[sorted_nodes] = s_arr[:, None]
    node_of = np.empty((NC, NSH), np.int64)
    node_of[owner[sorted_nodes.ravel()], slot[sorted_nodes.ravel()]] = sorted_nodes.ravel()
    row_of = owner * NSHP + slot                   # node -> table row

    # group tiles
    groups = [list(range(g, min(g + TPG, TILES))) for g in range(0, TILES, TPG)]

    # per (core, slot) lo/hi source-row lists
    ecore = owner[dst]
    eslot = slot[dst]
    esrow = row_of[src]
    lo_lists = [[None] * NSHP for _ in range(NC)]
    hi_lists = [[None] * NSHP for _ in range(NC)]
    for c in range(NC):
        m = ecore == c
        sl = eslot[m]
        rows = esrow[m]
        o = np.argsort(sl, kind="stable")
        sl = sl[o]
        rows = rows[o]
        counts = np.bincount(sl, minlength=NSHP)
        starts = np.concatenate([[0], np.cumsum(counts)])
        for p in range(NSHP):
            r = rows[starts[p]:starts[p + 1]]
            ml = r[r < HI_BASE]
            mh = r[r > 32767]
            fx = r[(r >= HI_BASE) & (r <= 32767)]
            nl, nh = len(ml), len(mh)
            lo_e, hi_e = [], []
            for v in fx:
                if nl <= nh:
                    lo_e.append(v); nl += 1
                else:
                    hi_e.append(v); nh += 1
            lo_lists[c][p] = np.concatenate([ml, np.array(lo_e, np.int64)]) if (len(ml) + len(lo_e)) else np.empty(0, np.int64)
            hi_lists[c][p] = (np.concatenate([mh, np.array(hi_e, np.int64)]) - HI_BASE) if (len(mh) + len(hi_e)) else np.empty(0, np.int64)

    # per-tile maxima over all cores/slots
    D_lo = np.zeros(TILES, np.int64)
    D_hi = np.zeros(TILES, np.int64)
    for c in range(NC):
        ll = np.array([len(x) for x in lo_lists[c]])
        hh = np.array([len(x) for x in hi_lists[c]])
        D_lo = np.maximum(D_lo, ll.reshape(TILES, 128).max(1))
        D_hi = np.maximum(D_hi, hh.reshape(TILES, 128).max(1))

    DG_lo = np.array([max(D_lo[t] for t in g) for g in groups])
    DG_hi = np.array([max(D_hi[t] for t in g) for g in groups])
    R = DG_lo + DG_hi
    NGR = len(groups)

    slots_tot = int(sum(int(R[g]) * len(groups[g]) * 128 for g in range(NGR)))
    pad_eff = slots_tot * NC / E

    # per-core idx (wrapped int16) and mask
    def wrap_idx(flat):
        n = len(flat)
        assert n % 16 == 0
        w = np.asarray(flat, np.int16).reshape(-1, 16).T
        return np.tile(w, (8, 1))

    # col offsets
    idx_off = {}          # (g, part) -> (col_start_in_idxall/16 units, n_idx)
    mask_off = {}         # g -> col start in maskall
    ic = 0
    mc = 0
    for g in range(NGR):
        tg = len(groups[g])
        idx_off[(g, "lo")] = (ic, int(DG_lo[g]) * tg * 128)
        ic += int(DG_lo[g]) * tg * 128 // 16
        idx_off[(g, "hi")] = (ic, int(DG_hi[g]) * tg * 128)
        ic += int(DG_hi[g]) * tg * 128 // 16
        mask_off[g] = mc
        mc += int(R[g]) * tg
    IDXCOLS = ic
    MASKCOLS = mc

    percore = []
    for c in range(NC):
        idxall = np.zeros((128, IDXCOLS), np.int16)
        maskall = np.zeros((128, MASKCOLS), np.float32)
        for g in range(NGR):
            tg = len(groups[g])
            dlo, dhi = int(DG_lo[g]), int(DG_hi[g])
            for part, dpart, lists in (("lo", dlo, lo_lists[c]), ("hi", dhi, hi_lists[c])):
                if dpart == 0:
                    continue
                flat = np.empty(dpart * tg * 128, np.int64)
                i = 0
                for k in range(dpart):
                    for j, t in enumerate(groups[g]):
                        base = t * 128
                        kk = k if part == "lo" else dlo + k
                        mrow = mask_off[g] + kk * tg + j
                        for sl in range(128):
                            lst = lists[base + sl]
                            if k < len(lst):
                                flat[i] = lst[k]
                                maskall[sl, mrow] = 1.0
                            else:
                                # pad fetch: spread across the table — same-row
                                # fetches serialize on one HBM bank (14x slower)
                                flat[i] = ((base + sl) * 97 + k * 12289) % 32000
                            i += 1
                col0, nidx = idx_off[(g, part)]
                assert nidx == len(flat)
                idxall[:, col0:col0 + nidx // 16] = wrap_idx(flat)
        percore.append({"idx": idxall, "mask": maskall})

    meta = {
        "groups": groups, "DG_lo": DG_lo, "DG_hi": DG_hi, "R": R,
        "idx_off": idx_off, "mask_off": mask_off,
        "IDXCOLS": IDXCOLS, "MASKCOLS": MASKCOLS,
        "order": order, "owner": owner, "slot": slot, "row_of": row_of,
        "node_of": node_of,
        "pad_eff": pad_eff, "slots_tot": slots_tot,
        "lo_lists": lo_lists, "hi_lists": hi_lists,
    }
    return meta, percore


# ---------------------------------------------------------------------------
# numpy emulator of the device computation (for logic validation)
# ---------------------------------------------------------------------------
def emulate(inputs, meta, percore, n_layers=L):
    bf = lambda a: a.astype(ml_dtypes.bfloat16).astype(np.float32)
    x = np.asarray(inputs["x"], np.float32)
    W0 = np.asarray(inputs["W0"], np.float32)
    Ws = np.asarray(inputs["Ws"], np.float32)
    att_src = np.asarray(inputs["att_src"], np.float32)
    att_dst = np.asarray(inputs["att_dst"], np.float32)
    lin_w = np.asarray(inputs["lin_w"], np.float32)
    lin_b = np.asarray(inputs["lin_b"], np.float32)

    groups = meta["groups"]
    DG_lo, DG_hi, R = meta["DG_lo"], meta["DG_hi"], meta["R"]
    order, row_of = meta["order"], meta["row_of"]
    NGR = len(groups)

    # per-core x slabs (rows = slots)
    Xc = np.zeros((NC, NSHP, IN), np.float32)
    node_of = np.zeros((NC, NSHP), np.int64)
    for c in range(NC):
        nodes = meta["node_of"][c]
        node_of[c, :NSH] = nodes
        Xc[c, :NSH] = bf(x[nodes])

    outfin = np.zeros((NC, NSHP, HID), np.float32)
    for l in range(n_layers):
        W = W0 if l == 0 else Ws[l - 1]
        Wext = np.concatenate([W, (W @ att_src[l])[:, None], (W @ att_dst[l])[:, None]], 1)
        Wext = bf(Wext)
        asrc_b = bf(att_src[l])

        # dense: h rows + alphas (bf16 rounded like device hrows_ext)
        HE = np.zeros((NC, NSHP, 130), np.float32)
        for c in range(NC):
            HE[c] = bf(Xc[c] @ Wext)

        # table: core blocks stacked
        table = bf(np.concatenate([HE[c][:, :128] for c in range(NC)], 0))  # [TBL,128]

        for c in range(NC):
            he = HE[c]
            as_own = he[:, 128]
            ad_own = he[:, 129]
            zs = as_own + ad_own
            wself = np.exp(np.where(zs > 0, zs, NEG * zs))
            mask = percore[c]["mask"]
            idxall = percore[c]["idx"]
            rows_out = np.zeros((NSHP, HID), np.float32)
            for g in range(NGR):
                tg = len(groups[g])
                dlo, dhi = int(DG_lo[g]), int(DG_hi[g])
                rg = int(R[g])
                # reconstruct gathered buffer from idx
                gbuf = np.zeros((128, rg * tg, 128), np.float32)
                for part, dpart, base_k, tb in (("lo", dlo, 0, 0), ("hi", dhi, dlo, HI_BASE)):
                    if dpart == 0:
                        continue
                    col0, nidx = meta["idx_off"][(g, part)]
                    wrapped = idxall[:16, col0:col0 + nidx // 16]
                    flat = wrapped.T.reshape(-1)          # undo wrap
                    rows = flat.astype(np.int64) + tb
                    gg = table[rows]                       # [n, 128]
                    gg = gg.reshape(dpart, tg, 128, 128)   # [k, j, sl, f]
                    for k in range(dpart):
                        for j in range(tg):
                            gbuf[:, (base_k + k) * tg + j, :] = gg[k, j]
                # dot
                tmp = bf(gbuf * asrc_b[None, None, :])
                asg = tmp.sum(2)                           # [128, rg*tg]
                # z = asg + ad (per tile j)
                ad_g = np.stack([ad_own[t * 128:(t + 1) * 128] for t in groups[g]], 1)  # [128, tg]
                z = asg.reshape(128, rg, tg) + ad_g[:, None, :]
                z = z.reshape(128, rg * tg)
                zl = np.where(z > 0, z, NEG * z)
                ew = np.exp(zl)
                m = mask[:, meta["mask_off"][g]:meta["mask_off"][g] + rg * tg]
                ewm = ew * m
                S = ewm.reshape(128, rg, tg).sum(1)        # [128, tg]
                ws_g = np.stack([wself[t * 128:(t + 1) * 128] for t in groups[g]], 1)
                rec = 1.0 / (S + SEPS + ws_g)
                ewn = bf(ewm * np.repeat(rec[:, None, :], rg, 1).reshape(128, rg * tg))
                gw = bf(gbuf * ewn[:, :, None])
                pa = gw.reshape(128, rg, tg, 128).sum(1)   # [128, tg, 128]
                wsn = ws_g * rec
                for j, t in enumerate(groups[g]):
                    h_own = he[t * 128:(t + 1) * 128, :128]
                    rows_out[t * 128:(t + 1) * 128] = pa[:, j] + h_own * wsn[:, j:j + 1]
            rows_out[NSH:] = 0.0
            HE[c, :, :128] = rows_out                      # reuse: pre-pairnorm rows
        # pairnorm stats (global)
        allrows = np.stack([bf(HE[c][:, :128]) for c in range(NC)])   # bf16 slab
        fsum = allrows.sum((0, 1))
        sq = (bf(allrows * allrows)).sum()
        mu = fsum / N
        var = PEPS + sq / N - (mu * mu).sum()
        invd = 1.0 / np.sqrt(var)
        for c in range(NC):
            xx = bf(HE[c][:, :128])
            g = (xx - mu[None, :]) * invd
            from scipy.special import erf
            gel = 0.5 * g * (1 + erf(g / np.sqrt(2)))
            gel[NSH:] = 0.0
            Xc[c] = bf(gel)
            outfin[c] += bf(Xc[c]) @ bf(lin_w[l * HID:(l + 1) * HID])

    out = np.empty((N, HID), np.float32)
    for c in range(NC):
        out[node_of[c, :NSH]] = outfin[c, :NSH] + lin_b[None, :]
    return out


def build(nc, meta, n_layers=L, sim_safe=False):
    groups = meta["groups"]
    DG_lo, DG_hi, R = meta["DG_lo"], meta["DG_hi"], meta["R"]
    idx_off, mask_off = meta["idx_off"], meta["mask_off"]
    IDXCOLS, MASKCOLS = meta["IDXCOLS"], meta["MASKCOLS"]
    NGR = len(groups)
    RMAX = int(R.max())
    TMPW = max(RMAX * TPG * 128, NSHP)
    GELU = AF.Sigmoid if sim_safe else AF.Gelu

    xT_in = nc.dram_tensor("xT", [128, NSHP], BF16, kind="ExternalInput")
    idx_in = nc.dram_tensor("idx", [128, IDXCOLS], I16, kind="ExternalInput")
    mask_in = nc.dram_tensor("mask", [128, MASKCOLS], BF16, kind="ExternalInput")
    wext_in = nc.dram_tensor("Wext", [n_layers, 128, 130], BF16, kind="ExternalInput")
    asrc_in = nc.dram_tensor("asrc", [n_layers, 128, 128], BF16, kind="ExternalInput")
    linw_in = nc.dram_tensor("linw", [n_layers, 128, 128], BF16, kind="ExternalInput")
    linbr_in = nc.dram_tensor("linbr", [128, 128], F32, kind="ExternalInput")
    identb_in = nc.dram_tensor("identb", [128, 128], BF16, kind="ExternalInput")
    ones_in = nc.dram_tensor("ones", [128, 128], F32, kind="ExternalInput")
    padm_in = nc.dram_tensor("padm", [128, 1], F32, kind="ExternalInput")
    y_out = nc.dram_tensor("y", [NSHP, 128], F32, kind="ExternalOutput")

    ag_in = nc.dram_tensor("ag_in", [NSHP, 128], BF16)
    table = nc.dram_tensor("table", [TBL, 128], BF16, addr_space="Shared")
    st_in = nc.dram_tensor("st_in", [128, 2], F32)
    st_out = nc.dram_tensor("st_out", [NC * 128, 2], F32, addr_space="Shared")

    RG = [list(range(NC))]

    with tile.TileContext(nc) as tc, ExitStack() as ctx:
        P = ctx.enter_context(tc.tile_pool(name="persist", bufs=1))
        xT = P.tile([128, NSHP], BF16, tag="xT")
        he = P.tile([128, TILES * 130], BF16, tag="he")
        outfin = P.tile([128, NSHP], F32, tag="outfin")
        idx_sb = P.tile([128, IDXCOLS], I16, tag="idx")
        mask_sb = P.tile([128, MASKCOLS], BF16, tag="mask")
        wext_sb = P.tile([128, n_layers * 130], BF16, tag="Wext")
        asrc_sb = P.tile([128, n_layers * 128], BF16, tag="asrc")
        linw_sb = P.tile([128, n_layers * 128], BF16, tag="linw")
        linbr_sb = P.tile([128, 128], F32, tag="linbr")
        identb = P.tile([128, 128], BF16, tag="identb")
        ones_sb = P.tile([128, 128], F32, tag="ones")
        padm_sb = P.tile([128, 1], F32, tag="padm")
        wself = P.tile([128, TILES], F32, tag="wself")
        stp = P.tile([128, 2], F32, tag="stp")
        gtmp = P.tile([128, 16], F32, tag="gtmp")
        gstat = P.tile([128, 2], F32, tag="gstat")

        nc.sync.dma_start(idx_sb[:], idx_in.ap())
        nc.sync.dma_start(mask_sb[:], mask_in.ap())
        nc.sync.dma_start(wext_sb[:].rearrange("a (l b) -> a l b", b=130), wext_in.ap().rearrange("l a b -> a l b"))
        nc.sync.dma_start(asrc_sb[:].rearrange("a (l b) -> a l b", b=128), asrc_in.ap().rearrange("l a b -> a l b"))
        nc.sync.dma_start(linw_sb[:].rearrange("a (l b) -> a l b", b=128), linw_in.ap().rearrange("l a b -> a l b"))
        nc.sync.dma_start(linbr_sb[:], linbr_in.ap())
        nc.sync.dma_start(identb[:], identb_in.ap())
        nc.sync.dma_start(ones_sb[:], ones_in.ap())
        nc.sync.dma_start(padm_sb[:], padm_in.ap())
        nc.sync.dma_start(xT[:], xT_in.ap())

        PD = ctx.enter_context(tc.tile_pool(name="pd", bufs=2, space="PSUM"))
        PT = ctx.enter_context(tc.tile_pool(name="pt", bufs=2, space="PSUM"))
        PA = ctx.enter_context(tc.tile_pool(name="pa", bufs=2, space="PSUM"))
        PM = ctx.enter_context(tc.tile_pool(name="pm", bufs=2, space="PSUM"))
        GB = ctx.enter_context(tc.tile_pool(name="gb", bufs=2))
        WK = ctx.enter_context(tc.tile_pool(name="wk", bufs=1))
        SC = ctx.enter_context(tc.tile_pool(name="sc", bufs=3))
        SS = ctx.enter_context(tc.tile_pool(name="ss", bufs=4))

        he3 = he[:].rearrange("p (t c) -> p t c", c=130)

        for l in range(n_layers):
            wxl = wext_sb[:, l * 130:(l + 1) * 130]
            asl = asrc_sb[:, l * 128:(l + 1) * 128]
            lwl = linw_sb[:, l * 128:(l + 1) * 128]

            # ---- dense: h rows + alpha_src/alpha_dst per tile
            for t in range(TILES):
                pd = PD.tile([128, 130], F32, tag="pd")
                nc.tensor.matmul(pd[:], xT[:, t * 128:(t + 1) * 128], wxl,
                                 start=True, stop=True)
                nc.scalar.activation(he[:, t * 130:(t + 1) * 130], pd[:], AF.Copy)

            # ---- self weight: wself = exp(LR(as+ad))
            zs = SC.tile([128, TILES], F32, tag="zs")
            nc.vector.tensor_tensor(zs[:], he3[:, :, 128], he3[:, :, 129], op=OP.add)
            zs2 = SC.tile([128, TILES], F32, tag="zs2")
            nc.vector.scalar_tensor_tensor(out=zs2[:], in0=zs[:], scalar=NEG,
                                           in1=zs[:], op0=OP.mult, op1=OP.max)
            nc.scalar.activation(wself[:], zs2[:], AF.Exp)

            # ---- AllGather h rows
            nc.sync.dma_start(ag_in.ap().rearrange("(t p) f -> p t f", p=128),
                              he3[:, :, 0:128])
            nc.gpsimd.collective_compute(
                "AllGather", OP.bypass, replica_groups=RG,
                ins=[ag_in.ap()], outs=[table.ap()])

            # ---- final-linear increment for the PREVIOUS layer's output
            # (xT currently holds outs[l-1]; runs in the AG shadow)
            if l > 0:
                lwp = linw_sb[:, (l - 1) * 128:l * 128]
                for t in range(TILES):
                    pf = PM.tile([128, 128], F32, tag="pm")
                    nc.tensor.matmul(pf[:], xT[:, t * 128:(t + 1) * 128], lwp,
                                     start=True, stop=True)
                    tsl = slice(t * 128, (t + 1) * 128)
                    if l == 1:
                        nc.vector.tensor_copy(outfin[:, tsl], pf[:])
                    else:
                        nc.vector.tensor_tensor(outfin[:, tsl], outfin[:, tsl], pf[:], op=OP.add)

            # ---- aggregation per group
            for g in range(NGR):
                tg = len(groups[g])
                dlo, dhi = int(DG_lo[g]), int(DG_hi[g])
                rg = dlo + dhi
                ncols = rg * tg
                gbuf = GB.tile([128, RMAX * TPG * 128], BF16, tag="gb")
                g3 = gbuf[:].rearrange("p (c f) -> p c f", f=128)
                if dlo:
                    c0, nidx = idx_off[(g, "lo")]
                    nc.gpsimd.dma_gather(
                        g3[:, :dlo * tg, :], table.ap()[:32768, :],
                        idx_sb[:, c0:c0 + nidx // 16], nidx, nidx, 128,
                        single_packet=False)
                if dhi:
                    c0, nidx = idx_off[(g, "hi")]
                    nc.gpsimd.dma_gather(
                        g3[:, dlo * tg:ncols, :], table.ap()[HI_BASE:TBL, :],
                        idx_sb[:, c0:c0 + nidx // 16], nidx, nidx, 128,
                        single_packet=False)

                wk = WK.tile([128, TMPW], BF16, tag="wk")
                w3 = wk[:].rearrange("p (c f) -> p c f", f=128)
                # dot: asg[p, c] = sum_f gbuf[p,c,f]*a[f]
                a_b = asl.unsqueeze(1).broadcast_to([128, ncols, 128])
                nc.vector.tensor_tensor(w3[:, :ncols, :], g3[:, :ncols, :], a_b, op=OP.mult)
                asg = SC.tile([128, RMAX * TPG], F32, tag="asg")
                nc.vector.tensor_reduce(out=asg[:, :ncols], in_=w3[:, :ncols, :],
                                        axis=AX.X, op=OP.add)
                # z = asg + ad_bcast ; LeakyReLU ; exp
                ad_b = he3[:, groups[g][0]:groups[g][0] + tg, 129] \
                    .unsqueeze(1).broadcast_to([128, rg, tg])
                zg = SC.tile([128, RMAX * TPG], F32, tag="zg")
                nc.vector.tensor_tensor(zg[:, :ncols].rearrange("p (k j) -> p k j", j=tg),
                                        asg[:, :ncols].rearrange("p (k j) -> p k j", j=tg),
                                        ad_b, op=OP.add)
                zg2 = SC.tile([128, RMAX * TPG], F32, tag="zg2")
                nc.vector.scalar_tensor_tensor(out=zg2[:, :ncols], in0=zg[:, :ncols],
                                               scalar=NEG, in1=zg[:, :ncols],
                                               op0=OP.mult, op1=OP.max)
                ew = SC.tile([128, RMAX * TPG], F32, tag="ew")
                nc.scalar.activation(ew[:, :ncols], zg2[:, :ncols], AF.Exp)
                # masked weights + per-tile sums
                ewm = SC.tile([128, RMAX * TPG], F32, tag="ewm")
                mo = mask_off[g]
                nc.vector.tensor_tensor(ewm[:, :ncols], ew[:, :ncols],
                                        mask_sb[:, mo:mo + ncols], op=OP.mult)
                S = SS.tile([128, TPG], F32, tag="S")
                nc.vector.tensor_reduce(out=S[:, :tg],
                                        in_=ewm[:, :ncols].rearrange("p (k j) -> p j k", j=tg),
                                        axis=AX.X, op=OP.add)
                ws_g = wself[:, groups[g][0]:groups[g][0] + tg]
                Sp = SS.tile([128, TPG], F32, tag="Sp")
                nc.vector.scalar_tensor_tensor(out=Sp[:, :tg], in0=S[:, :tg], scalar=SEPS,
                                               in1=ws_g, op0=OP.add, op1=OP.add)
                rec = SS.tile([128, TPG], F32, tag="rec")
                nc.vector.reciprocal(rec[:, :tg], Sp[:, :tg])
                # normalized weights, expanded apply
                ewn = SC.tile([128, RMAX * TPG], BF16, tag="ewn")
                rec_b = rec[:, :tg].unsqueeze(1).broadcast_to([128, rg, tg])
                nc.vector.tensor_tensor(ewn[:, :ncols].rearrange("p (k j) -> p k j", j=tg),
                                        ewm[:, :ncols].rearrange("p (k j) -> p k j", j=tg),
                                        rec_b, op=OP.mult)
                ewn_e = ewn[:, :ncols].unsqueeze(-1).broadcast_to([128, ncols, 128])
                nc.vector.tensor_tensor(w3[:, :ncols, :], g3[:, :ncols, :], ewn_e, op=OP.mult)
                # accumulate rounds
                pa = PA.tile([128, TPG * 128], F32, tag="pa")
                for k in range(rg):
                    nc.tensor.matmul(pa[:, :tg * 128], identb[:],
                                     wk[:, k * tg * 128:(k + 1) * tg * 128],
                                     start=(k == 0), stop=(k == rg - 1))
                # self term + normalize-free combine
                wsn = SS.tile([128, TPG], F32, tag="wsn")
                nc.vector.tensor_tensor(wsn[:, :tg], ws_g, rec[:, :tg], op=OP.mult)
                hslab = he3[:, groups[g][0]:groups[g][0] + tg, 0:128]
                selft = SC.tile([128, TPG * 128], F32, tag="selft")
                wsn_b = wsn[:, :tg].unsqueeze(-1).broadcast_to([128, tg, 128])
                nc.vector.tensor_tensor(selft[:, :tg * 128].rearrange("p (j f) -> p j f", f=128),
                                        hslab, wsn_b, op=OP.mult)
                rows = SC.tile([128, TPG * 128], BF16, tag="rows")
                nc.vector.scalar_tensor_tensor(out=rows[:, :tg * 128], in0=pa[:, :tg * 128],
                                               scalar=1.0, in1=selft[:, :tg * 128],
                                               op0=OP.mult, op1=OP.add)
                if groups[g][-1] == TILES - 1:
                    lastj = (TILES - 1) - groups[g][0]
                    nc.vector.tensor_scalar_mul(rows[:, lastj * 128:(lastj + 1) * 128],
                                                rows[:, lastj * 128:(lastj + 1) * 128],
                                                padm_sb[:])
                # transpose into xT slab (pre-pairnorm)
                for j, t in enumerate(groups[g]):
                    pt = PT.tile([128, 128], BF16, tag="pt")
                    nc.tensor.transpose(pt[:], rows[:, j * 128:(j + 1) * 128], identb[:])
                    nc.scalar.activation(xT[:, t * 128:(t + 1) * 128], pt[:], AF.Copy)

            # ---- pairnorm stats
            fsum = SS.tile([128, 1], F32, tag="fsum")
            nc.vector.tensor_reduce(out=fsum[:], in_=xT[:], axis=AX.X, op=OP.add)
            wk = WK.tile([128, TMPW], BF16, tag="wk")
            sqc = SS.tile([128, 1], F32, tag="sqc")
            nc.scalar.activation(wk[:, :NSHP], xT[:], AF.Square, accum_out=sqc[:])
            nc.vector.tensor_copy(stp[:, :1], fsum[:])
            nc.vector.tensor_copy(stp[:, 1:2], sqc[:])
            nc.sync.dma_start(st_in.ap(), stp[:])
            nc.gpsimd.collective_compute(
                "AllGather", OP.bypass, replica_groups=RG,
                ins=[st_in.ap()], outs=[st_out.ap()])
            nc.sync.dma_start(gtmp[:].rearrange("p (r c) -> p r c", c=2),
                              st_out.ap().rearrange("(r p) c -> p r c", p=128))
            nc.vector.tensor_reduce(out=gstat[:],
                                    in_=gtmp[:].rearrange("p (r c) -> p c r", c=2),
                                    axis=AX.X, op=OP.add)
            mu = SS.tile([128, 1], F32, tag="mu")
            nc.vector.tensor_scalar_mul(mu[:], gstat[:, :1], 1.0 / N)
            st2 = SS.tile([128, 2], F32, tag="st2")
            nc.vector.tensor_copy(st2[:, :1], gstat[:, 1:2])
            nc.vector.tensor_tensor(st2[:, 1:2], mu[:], mu[:], op=OP.mult)
            p2 = PM.tile([128, 128], F32, tag="pm")
            nc.tensor.matmul(p2[:1, :2], ones_sb[:, :1], st2[:], start=True, stop=True)
            tot = SS.tile([1, 2], F32, tag="tot")
            nc.vector.tensor_copy(tot[:], p2[:1, :2])
            v3 = SS.tile([1, 1], F32, tag="v3")
            nc.vector.tensor_scalar(v3[:], tot[:, :1], 1.0 / N, PEPS,
                                    op0=OP.mult, op1=OP.add)
            v4 = SS.tile([1, 1], F32, tag="v4")
            nc.vector.tensor_tensor(v4[:], v3[:], tot[:, 1:2], op=OP.subtract)
            den = SS.tile([1, 1], F32, tag="den")
            nc.scalar.activation(den[:], v4[:], AF.Sqrt)
            invd = SS.tile([1, 1], F32, tag="invd")
            nc.vector.reciprocal(invd[:], den[:])
            pb1 = PM.tile([128, 128], F32, tag="pm")
            nc.tensor.matmul(pb1[:, :1], ones_sb[:1, :], invd[:], start=True, stop=True)
            invdr = SS.tile([128, 1], F32, tag="invdr")
            nc.vector.tensor_copy(invdr[:], pb1[:, :1])
            nms = SS.tile([128, 1], F32, tag="nms")
            nc.vector.tensor_scalar(nms[:], mu[:], invdr[:], -1.0,
                                    op0=OP.mult, op1=OP.mult)

            # ---- pairnorm + gelu fused on ACT (in-place on xT)
            for ch0 in range(0, NSHP, 512):
                chsz = min(512, NSHP - ch0)
                csl = slice(ch0, ch0 + chsz)
                nc.scalar.activation(xT[:, csl], xT[:, csl], GELU,
                                     bias=nms[:], scale=invdr[:])

        # ---- final-linear for last layer + lin_b + output
        lwl = linw_sb[:, (n_layers - 1) * 128:n_layers * 128]
        for t in range(TILES):
            pf = PM.tile([128, 128], F32, tag="pm")
            nc.tensor.matmul(pf[:], xT[:, t * 128:(t + 1) * 128], lwl,
                             start=True, stop=True)
            tsl = slice(t * 128, (t + 1) * 128)
            if n_layers == 1:
                nc.vector.tensor_copy(outfin[:, tsl], pf[:])
            else:
                nc.vector.tensor_tensor(outfin[:, tsl], outfin[:, tsl], pf[:], op=OP.add)
        lb_b = linbr_sb[:].unsqueeze(1).broadcast_to([128, TILES, 128])
        nc.vector.tensor_tensor(outfin[:].rearrange("p (t f) -> p t f", f=128),
                                outfin[:].rearrange("p (t f) -> p t f", f=128),
                                lb_b, op=OP.add)
        nc.sync.dma_start(y_out.ap().rearrange("(t p) f -> p t f", p=128),
                          outfin[:].rearrange("p (t f) -> p t f", f=128))

    return nc


def make_inputs(inputs, meta, percore, n_layers=L):
    x = np.asarray(inputs["x"], np.float32)
    W0 = np.asarray(inputs["W0"], np.float32)
    Ws = np.asarray(inputs["Ws"], np.float32)
    att_src = np.asarray(inputs["att_src"], np.float32)
    att_dst = np.asarray(inputs["att_dst"], np.float32)
    lin_w = np.asarray(inputs["lin_w"], np.float32)
    lin_b = np.asarray(inputs["lin_b"], np.float32)
    node_of = meta["node_of"]

    wext = np.zeros((n_layers, 128, 130), np.float32)
    for l in range(n_layers):
        W = W0 if l == 0 else Ws[l - 1]
        wext[l, :, :128] = W
        wext[l, :, 128] = W @ att_src[l]
        wext[l, :, 129] = W @ att_dst[l]
    wext = wext.astype(ml_dtypes.bfloat16)
    asrc = np.stack([np.tile(att_src[i], (128, 1)) for i in range(n_layers)]).astype(ml_dtypes.bfloat16)
    linw = np.stack([lin_w[i * HID:(i + 1) * HID] for i in range(n_layers)]).astype(ml_dtypes.bfloat16)
    linbr = np.tile(lin_b, (128, 1)).astype(np.float32)
    identb = np.eye(128, dtype=ml_dtypes.bfloat16)
    ones = np.ones((128, 128), np.float32)
    padm = np.zeros((128, 1), np.float32)
    padm[:NSH - (TILES - 1) * 128] = 1.0

    in_maps = []
    for c in range(NC):
        nodes = node_of[c]
        xT = np.zeros((128, NSHP), ml_dtypes.bfloat16)
        xT[:, :NSH] = x[nodes].astype(ml_dtypes.bfloat16).T
        in_maps.append({
            "xT": xT, "idx": percore[c]["idx"],
            "mask": percore[c]["mask"].astype(ml_dtypes.bfloat16),
            "Wext": wext, "asrc": asrc, "linw": linw, "linbr": linbr,
            "identb": identb, "ones": ones, "padm": padm,
        })
    return in_maps


def assemble_output(results, meta):
    node_of = meta["node_of"]
    out = np.empty((N, HID), np.float32)
    for c in range(NC):
        out[node_of[c]] = results[c]["y"][:NSH]
    return out


_CACHE = {}


def _get_compiled(edge_key, edge_index):
    if edge_key not in _CACHE:
        meta, percore = preprocess(edge_index)
        nc = bacc.Bacc("TRN2", target_bir_lowering=False, debug=False,
                       num_devices=NC)
        build(nc, meta, n_layers=L, sim_safe=False)
        nc.compile()
        _CACHE[edge_key] = (nc, meta, percore)
    return _CACHE[edge_key]


def kernel(**inputs):
    from concourse.bass_utils import run_bass_kernel_spmd
    edge_index = np.asarray(inputs["edge_index"])
    edge_key = hash(edge_index.tobytes())
    nc, meta, percore = _get_compiled(edge_key, edge_index)
    in_maps = make_inputs(inputs, meta, percore, n_layers=L)
    res = run_bass_kernel_spmd(nc, in_maps, list(range(NC)))
    return assemble_output(res.results, meta)


if __name__ == "__main__":
    import sys, time, pickle, os
    sys.path.insert(0, "/root/problem")
    import jax
    import reference
    cpu = jax.devices("cpu")[0]
    with jax.default_device(cpu):
        inputs = {k: np.asarray(v) for k, v in reference.setup_inputs().items()}
    t0 = time.time()
    meta, percore = preprocess(inputs["edge_index"])
    print("preprocess:", round(time.time() - t0, 1), "s; pad_eff:", round(meta["pad_eff"], 3),
          "slots_tot:", meta["slots_tot"], "R:", meta["R"].tolist())
    if "--emulate" in sys.argv:
        t0 = time.time()
        got = emulate(inputs, meta, percore)
        print("emulate:", round(time.time() - t0, 1), "s")
        with jax.default_device(cpu):
            exp = np.asarray(reference.reference(**inputs))
        rel = np.linalg.norm(got - exp) / np.linalg.norm(exp)
        print("emulator rel-l2 vs reference:", rel)
    if "--build" in sys.argv:
        t0 = time.time()
        nc = bacc.Bacc("TRN2", target_bir_lowering=False, debug=False, num_devices=NC)
        build(nc, meta, n_layers=L, sim_safe=False)
        nc.compile()
        print("build+compile:", round(time.time() - t0, 1), "s")
        f = nc.m.functions[0]
        print("instructions:", sum(len(b.instructions) for b in f.blocks))
